# revision 26
# baseline (speedup 1.0000x reference)
"""Trainium2 Bass kernel for nn_CrossAttention (B=4, N=2048, E=768, H=8).

Sharding: 8 cores = 4 batches x 2 head-groups (4 heads of 96 dims each).
Each core computes its batch's attention for its 4 heads plus the partial
output projection; the host sums the two head-group partials per batch and
adds bo.

Per-core dataflow (all feature-major, no on-device transposes):
  K^T_h [96,2048] = Wk_h @ x_kv^T        (lhsT = Wk^T e-tiles, rhs = x_kv^T)
  Q^T_h [96,2048] = Wq_h @ x_q^T
  V     [128tok,4,97] tiles (col 96 = ones -> rowsums ride along matmul)
  S^T   [128kv,1024q] = K_h @ Q_h^T      (lhsT = K^T slice, rhs = Q^T)
  P^T   = exp(S^T/sqrt(768))             (ScalarE, PSUM->SBUF)
  O^T   [97,1024] += V_ext^T @ P^T       (lhsT = V tile, rhs = P^T)
  attn^T = O^T[0:96] * recip(bcast(O^T[96]))   (GpSimd bcast + DVE)
  out   [128q,768] += attn^T_h.T @ Wo^T_h  (partial; host adds group pairs + bo)

v2: single software-pipelined instruction stream.  The attention inner loop
is paced by ScalarE's exp; projection and output-projection matmul groups
are drained from a background queue inside the loop so the PE never idles
(keeps HAM at full clock).  Normalization uses a GpSimd partition-broadcast
of the rowsum row instead of PE ones-matmuls, freeing PSUM and PE cycles.
x tiles are loaded per (e, half) chunk so the first projection matmul can
start ~2us into the kernel.
"""

import os
import sys
import types

import numpy as np

# ---------------------------------------------------------------------------
# NTFF profile hook (the agent image's antenv lacks axon_hooks; degrade OK)
# ---------------------------------------------------------------------------
def _install_ntff_hook():
    if "antenv.axon_hooks" in sys.modules:
        return
    try:
        hooks = types.ModuleType("antenv.axon_hooks")
        hooks._hook = None
        hooks.set_axon_ntff_profile_hook = lambda h: setattr(hooks, "_hook", h)
        hooks.get_axon_ntff_profile_hook = lambda: hooks._hook
        sys.modules["antenv.axon_hooks"] = hooks
        import antenv

        antenv.axon_hooks = hooks
        from trn_agent_boot.trn_boot import _ntff_profile_via_ctypes

        so = "/opt/axon/libaxon_pjrt.so"
        if os.path.exists(so):
            hooks.set_axon_ntff_profile_hook(_ntff_profile_via_ctypes(so))
    except Exception:
        pass


_install_ntff_hook()

import concourse.bacc as bacc
import concourse.tile as tile
import concourse.mybir as mybir
from concourse import bass_utils
from concourse.alu_op_type import AluOpType

F32 = mybir.dt.float32
BF16 = mybir.dt.bfloat16

B = 4
NQ = 2048
NKV = 2048
E = 768
H_LOCAL = 4  # heads per core
HD = 96  # head dim
D = H_LOCAL * HD  # 384 local proj dim
ET = E // 128  # 6 contraction tiles
KV_T = NKV // 128  # 16 kv tiles
INV_SQRT_E = 1.0 / float(np.sqrt(np.float32(E)))

# rowsum broadcast mode: "gpsimd" (idle engine) or "pe" (ones-matmul)
BCAST = os.environ.get("KERNEL_BCAST", "gpsimd")


def build_nc():
    nc = bacc.Bacc("TRN2", target_bir_lowering=False, debug=False)

    xq_t = nc.dram_tensor("xq_t", [E, NQ], BF16, kind="ExternalInput")
    xkv_t = nc.dram_tensor("xkv_t", [E, NKV], BF16, kind="ExternalInput")
    wq_t = nc.dram_tensor("wq_t", [E, D], BF16, kind="ExternalInput")
    wk_t = nc.dram_tensor("wk_t", [E, D], BF16, kind="ExternalInput")
    wv_t = nc.dram_tensor("wv_t", [E, D], BF16, kind="ExternalInput")
    wo_t = nc.dram_tensor("wo_t", [D, E], BF16, kind="ExternalInput")
    bq = nc.dram_tensor("bq", [D], F32, kind="ExternalInput")
    bk = nc.dram_tensor("bk", [D], F32, kind="ExternalInput")
    bv = nc.dram_tensor("bv", [D], F32, kind="ExternalInput")
    out = nc.dram_tensor("out", [NQ, E], F32, kind="ExternalOutput")

    with tile.TileContext(nc) as tc:
        with (
            nc.allow_low_precision(reason="bf16 matmul operands"),
            tc.tile_pool(name="persist", bufs=1) as persist,
            tc.tile_pool(name="work", bufs=1) as work,
            tc.tile_pool(name="psum_pf", bufs=2, space="PSUM") as ppf,
            tc.tile_pool(name="psum_s", bufs=2, space="PSUM") as pps,
            tc.tile_pool(name="psum_o", bufs=1, space="PSUM") as ppo,
        ):
            # ---------------- persistent SBUF ----------------
            xkv_sb = persist.tile([128, ET, 2, 1024], BF16)
            xq_sb = persist.tile([128, ET, 2, 1024], BF16)
            wq_sb = persist.tile([128, ET, D], BF16, tag="wq")
            wk_sb = persist.tile([128, ET, D], BF16, tag="wk")
            wv_sb = persist.tile([128, ET, D], BF16, tag="wv")
            wo_sb = persist.tile([HD, H_LOCAL, E], BF16)
            bq_sb = persist.tile([128, 3], F32)
            bk_sb = persist.tile([128, 3], F32)
            bv_sb = persist.tile([128, D], F32)
            KT = persist.tile([HD, H_LOCAL, NKV], BF16)
            QT = persist.tile([HD, H_LOCAL, NQ], BF16)
            V = persist.tile([128, KV_T, H_LOCAL, HD + 1], BF16)
            attn = persist.tile([HD, H_LOCAL, NQ], BF16)
            ones_f32 = persist.tile([128, HD], F32)
            ones_bf = persist.tile([1, HD], BF16)

            # ---------------- input DMAs (order = priority) ----------------
            nc.sync.dma_start(wk_sb[:], wk_t[:].rearrange("(t p) n -> p t n", p=128))
            nc.sync.dma_start(bq_sb[:], bq[:].rearrange("(j p) -> p j", p=128))
            nc.sync.dma_start(bk_sb[:], bk[:].rearrange("(j p) -> p j", p=128))
            nc.sync.dma_start(bv_sb[:], bv[:].partition_broadcast(128))
            for e in range(ET):
                nc.sync.dma_start(
                    xkv_sb[:, e, 0, :], xkv_t[e * 128 : (e + 1) * 128, 0:1024]
                )
            nc.sync.dma_start(wq_sb[:], wq_t[:].rearrange("(t p) n -> p t n", p=128))
            for e in range(ET):
                nc.sync.dma_start(
                    xq_sb[:, e, 0, :], xq_t[e * 128 : (e + 1) * 128, 0:1024]
                )
            nc.sync.dma_start(wv_sb[:], wv_t[:].rearrange("(t p) n -> p t n", p=128))
            for e in range(ET):
                nc.sync.dma_start(
                    xkv_sb[:, e, 1, :], xkv_t[e * 128 : (e + 1) * 128, 1024:2048]
                )
            for e in range(ET):
                nc.sync.dma_start(
                    xq_sb[:, e, 1, :], xq_t[e * 128 : (e + 1) * 128, 1024:2048]
                )
            nc.sync.dma_start(wo_sb[:], wo_t[:].rearrange("(h p) n -> p h n", p=HD))

            # ones column for rowsum-via-matmul + exp-table warm dummy
            nc.vector.memset(ones_f32[:], 1.0)
            nc.vector.tensor_copy(ones_bf[:], ones_f32[0:1, :])
            nc.vector.tensor_copy(
                V[:, :, :, HD : HD + 1],
                ones_f32[:, 0 : KV_T * H_LOCAL].rearrange(
                    "p (t h one) -> p t h one", t=KV_T, h=H_LOCAL, one=1
                ),
            )
            warm = work.tile([128, HD], BF16, tag="warm", bufs=1)
            nc.scalar.activation(
                warm[:], ones_f32[:], mybir.ActivationFunctionType.Exp, scale=0.5
            )

            # ---------------- compute group helpers ----------------
            def proj_col(x_sb, w_sb, b_sb, dst, c):
                # stacked projection: one 512-col chunk of x, all 4 heads.
                # 3 full-width (M=128) matmul groups over the flat 384 output
                # dims (25% fewer PE cycles than 4 per-head M=96 groups),
                # then 6 small DMAs shuffle the stacked rows into the
                # per-head [96, h, n] layout the S-matmuls consume.
                half, n = c // 2, c % 2
                stk = work.tile([128, 3, 512], BF16, tag="stk", bufs=2)
                for j in range(3):
                    ps = ppf.tile([128, 512], F32, tag="pf")
                    for e in range(ET):
                        nc.tensor.matmul(
                            ps[:],
                            w_sb[:, e, j * 128 : (j + 1) * 128],
                            x_sb[:, e, half, n * 512 : (n + 1) * 512],
                            start=(e == 0),
                            stop=(e == ET - 1),
                        )
                    nc.vector.tensor_scalar_add(
                        out=stk[:, j, :], in0=ps[:], scalar1=b_sb[:, j : j + 1]
                    )
                cs = c * 512
                for dst_sl, j, src_lo, src_hi in (
                    ((0, 96, 0), 0, 0, 96),
                    ((0, 32, 1), 0, 96, 128),
                    ((32, 96, 1), 1, 0, 64),
                    ((0, 64, 2), 1, 64, 128),
                    ((64, 96, 2), 2, 0, 32),
                    ((0, 96, 3), 2, 32, 128),
                ):
                    lo, hi, h = dst_sl
                    nc.sync.dma_start(
                        dst[lo:hi, h, cs : cs + 512], stk[src_lo:src_hi, j, :]
                    )

            def v_group(tg):
                half, t = tg // 8, tg % 8
                ps = ppf.tile([128, 512], F32, tag="pf")
                for e in range(ET):
                    nc.tensor.matmul(
                        ps[:, 0:D],
                        xkv_sb[:, e, half, t * 128 : (t + 1) * 128],
                        wv_sb[:, e, :],
                        start=(e == 0),
                        stop=(e == ET - 1),
                    )
                nc.vector.tensor_tensor(
                    out=V[:, tg, :, 0:HD],
                    in0=ps[:, 0:D].rearrange("p (h d) -> p h d", h=H_LOCAL),
                    in1=bv_sb.rearrange("p (h d) -> p h d", h=H_LOCAL),
                    op=AluOpType.add,
                )

            def outproj_group(qc, t):
                qt = qc * 8 + t
                fa = ppf.tile([128, 512], F32, tag="pf")
                fb = ppf.tile([128, 512], F32, tag="pf")
                for h in range(H_LOCAL):
                    nc.tensor.matmul(
                        fa[:],
                        attn[:, h, qt * 128 : (qt + 1) * 128],
                        wo_sb[:, h, 0:512],
                        start=(h == 0),
                        stop=(h == H_LOCAL - 1),
                    )
                    nc.tensor.matmul(
                        fb[:, 0:256],
                        attn[:, h, qt * 128 : (qt + 1) * 128],
                        wo_sb[:, h, 512:768],
                        start=(h == 0),
                        stop=(h == H_LOCAL - 1),
                    )
                ob = work.tile([128, E], F32, tag="ob", bufs=2)
                nc.vector.tensor_copy(ob[:, 0:512], fa[:])
                nc.vector.tensor_copy(ob[:, 512:768], fb[:, 0:256])
                nc.sync.dma_start(out[qt * 128 : (qt + 1) * 128, :], ob[:])

            # ---------------- upfront projections ----------------
            # One stacked K column and the two qc0 Q columns give every head
            # enough to start attention ~15us in; everything else becomes
            # background work drained inside the attention regions.
            proj_col(xkv_sb, wk_sb, bk_sb, KT, 0)
            proj_col(xq_sb, wq_sb, bq_sb, QT, 0)
            proj_col(xq_sb, wq_sb, bq_sb, QT, 1)
            for tg in range(4):
                v_group(tg)

            bg = [
                ("k", 1),
                ("v", 4),
                ("v", 5),
                ("k", 2),
                ("v", 6),
                ("v", 7),
                ("k", 3),
            ] + [("v", tg) for tg in range(8, KV_T)]
            bg_late = [("q", 2), ("q", 3)]

            def drain_bg(k):
                while k > 0 and bg:
                    item = bg.pop(0)
                    if item[0] == "v":
                        v_group(item[1])
                    elif item[0] == "k":
                        proj_col(xkv_sb, wk_sb, bk_sb, KT, item[1])
                    elif item[0] == "q":
                        proj_col(xq_sb, wq_sb, bq_sb, QT, item[1])
                    else:
                        _, qc, t = item
                        outproj_group(qc, t)
                    k -= 1

            # ---------------- attention regions ----------------
            def norm_chunk(qc, h, o_sb, col, width, attn_col=None):
                if attn_col is None:
                    attn_col = col
                # o_sb rows 0:96 = O^T, row 96 = rowsums for cols [col,col+width)
                # of this (qc,h) q-chunk.  Engines can't shift partitions, so
                # DMA the rowsum row down to partition 0, broadcast it across
                # 96 partitions on the idle GpSimd engine, then recip+mul.
                rs = work.tile([1, 1024], F32, tag="rsrow", bufs=2)
                nc.sync.dma_start(rs[0:1, 0:width], o_sb[HD : HD + 1, col : col + width])
                rb_raw = work.tile([HD, 1024], F32, tag="rbr", bufs=2)
                if BCAST == "gpsimd":
                    nc.gpsimd.partition_broadcast(
                        rb_raw[:, 0:width], rs[0:1, 0:width]
                    )
                else:
                    rs_bf = work.tile([1, 1024], BF16, tag="rsbf", bufs=2)
                    nc.vector.tensor_copy(rs_bf[0:1, 0:width], rs[0:1, 0:width])
                    for n in range(width // 512):
                        bc = ppf.tile([128, 512], F32, tag="pf")
                        nc.tensor.matmul(
                            bc[0:HD, :],
                            ones_bf[:],
                            rs_bf[:, n * 512 : (n + 1) * 512],
                            start=True,
                            stop=True,
                        )
                        nc.vector.tensor_copy(
                            rb_raw[:, n * 512 : (n + 1) * 512], bc[0:HD, :]
                        )
                rb_inv = work.tile([HD, 1024], F32, tag="rbi", bufs=2)
                rscr = work.tile([HD, 1024], F32, tag="rsc", bufs=2)
                nc.vector.reciprocal_approx_accurate(
                    out=rb_inv[:, 0:width],
                    in_=rb_raw[:, 0:width],
                    scratch=rscr[:, 0:width],
                )
                nc.vector.tensor_mul(
                    attn[:, h, qc * 1024 + attn_col : qc * 1024 + attn_col + width],
                    o_sb[0:HD, col : col + width],
                    rb_inv[:, 0:width],
                )

            def region(qc, h, bg_rate, split_norm=False):
                po = ppo.tile([HD + 1, 1024], F32, tag="o")
                for kv in range(KV_T):
                    s = pps.tile([128, 1024], F32, tag="s")
                    for n in range(2):
                        nc.tensor.matmul(
                            s[:, n * 512 : (n + 1) * 512],
                            KT[:, h, kv * 128 : (kv + 1) * 128],
                            QT[:, h, qc * 1024 + n * 512 : qc * 1024 + (n + 1) * 512],
                            start=True,
                            stop=True,
                        )
                    p = work.tile([128, 1024], BF16, tag="p", bufs=3)
                    nc.scalar.activation(
                        p[:], s[:], mybir.ActivationFunctionType.Exp, scale=INV_SQRT_E
                    )
                    for n in range(2):
                        nc.tensor.matmul(
                            po[:, n * 512 : (n + 1) * 512],
                            V[:, kv, h, :],
                            p[:, n * 512 : (n + 1) * 512],
                            start=(kv == 0),
                            stop=(kv == KV_T - 1),
                        )
                    num, den = bg_rate
                    if den and kv % den == den - 1:
                        drain_bg(num)
                o_sb = work.tile([HD + 1, 1024], F32, tag="osb", bufs=2)
                if split_norm:
                    # halve the tail latency: normalize cols 0:512 while the
                    # copy of cols 512:1024 is still in flight
                    nc.vector.tensor_copy(o_sb[:, 0:512], po[:, 0:512])
                    norm_chunk(qc, h, o_sb, 0, 512)
                    nc.vector.tensor_copy(o_sb[:, 512:1024], po[:, 512:1024])
                    norm_chunk(qc, h, o_sb, 512, 512)
                else:
                    nc.vector.tensor_copy(o_sb[:], po[:])
                    norm_chunk(qc, h, o_sb, 0, 1024)

            # qc0: heads 0-3 with proj background work
            region(0, 0, (1, 1))
            bg.extend(bg_late)
            region(0, 1, (1, 8))
            region(0, 2, (0, 0))
            region(0, 3, (0, 0))
            # qc0 output projection rides through the qc1 regions
            for t in range(8):
                bg.append(("op", 0, t))
            region(1, 0, (1, 8))
            region(1, 1, (1, 8))
            region(1, 2, (1, 8))

            # ---- final region (1,3): two 512-wide half-regions ----
            # The closing normalization chain (~5us of cross-engine latency)
            # cannot be hidden by the scheduler once all other work has
            # drained, so split the last region: half-a's normalization
            # hides under half-b's kv loop, and half-b's normalization hides
            # under half-a's output-projection tiles.
            def half_region(hb):
                cs = 1024 + hb * 512
                po = ppo.tile([HD + 1, 1024], F32, tag="o")
                for kv in range(KV_T):
                    s = pps.tile([128, 1024], F32, tag="s")
                    nc.tensor.matmul(
                        s[:, 0:512],
                        KT[:, 3, kv * 128 : (kv + 1) * 128],
                        QT[:, 3, cs : cs + 512],
                        start=True,
                        stop=True,
                    )
                    p = work.tile([128, 1024], BF16, tag="p", bufs=3)
                    nc.scalar.activation(
                        p[:, 0:512],
                        s[:, 0:512],
                        mybir.ActivationFunctionType.Exp,
                        scale=INV_SQRT_E,
                    )
                    nc.tensor.matmul(
                        po[:, 0:512],
                        V[:, kv, 3, :],
                        p[:, 0:512],
                        start=(kv == 0),
                        stop=(kv == KV_T - 1),
                    )
                    drains = (5, 11) if hb == 0 else (6, 10, 13)
                    if kv in drains and bg:
                        # leftover outproj tiles interleave with the thin
                        # kv loops to fill the ScalarE-paced PE slack
                        item = bg.pop(0)
                        outproj_group(item[1], item[2])
                o_sb = work.tile([HD + 1, 1024], F32, tag="osb", bufs=2)
                nc.vector.tensor_copy(o_sb[:, 0:512], po[:, 0:512])
                norm_chunk(1, 3, o_sb, 0, 512, attn_col=hb * 512)

            half_region(0)
            # queue the half-a output tiles; t0-t3 only need attn cols
            # 1024:1536 (half-a) of head 3
            bg = [("op", 1, t) for t in range(4)] + bg
            half_region(1)
            drain_bg(len(bg))
            for t in range(4, 8):
                outproj_group(1, t)

    nc.compile()
    return nc


_NC_CACHE = None


def kernel(x_query, x_kv, Wq, bq, Wk, bk, Wv, bv, Wo, bo):
    global _NC_CACHE
    x_query = np.asarray(x_query, dtype=np.float32)
    x_kv = np.asarray(x_kv, dtype=np.float32)
    Wq = np.asarray(Wq, dtype=np.float32)
    Wk = np.asarray(Wk, dtype=np.float32)
    Wv = np.asarray(Wv, dtype=np.float32)
    Wo = np.asarray(Wo, dtype=np.float32)
    bq = np.asarray(bq, dtype=np.float32)
    bk = np.asarray(bk, dtype=np.float32)
    bv = np.asarray(bv, dtype=np.float32)
    bo = np.asarray(bo, dtype=np.float32)

    if _NC_CACHE is None:
        _NC_CACHE = build_nc()
    nc = _NC_CACHE

    import ml_dtypes

    xdt_np = ml_dtypes.bfloat16

    in_maps = []
    for c in range(8):
        b, g = divmod(c, 2)
        sl = slice(g * D, (g + 1) * D)
        in_maps.append(
            {
                "xq_t": np.ascontiguousarray(x_query[b].T).astype(xdt_np),
                "xkv_t": np.ascontiguousarray(x_kv[b].T).astype(xdt_np),
                "wq_t": np.ascontiguousarray(Wq[sl, :].T).astype(xdt_np),
                "wk_t": np.ascontiguousarray(Wk[sl, :].T).astype(xdt_np),
                "wv_t": np.ascontiguousarray(Wv[sl, :].T).astype(xdt_np),
                "wo_t": np.ascontiguousarray(Wo[:, sl].T).astype(xdt_np),
                "bq": np.ascontiguousarray(bq[sl]),
                "bk": np.ascontiguousarray(bk[sl]),
                "bv": np.ascontiguousarray(bv[sl]),
            }
        )

    trace = bool(int(os.environ.get("KERNEL_TRACE", "0")))
    res = bass_utils.run_bass_kernel_spmd(
        nc, in_maps, core_ids=list(range(8)), trace=trace
    )
    if trace:
        kernel.last_exec_time_ns = res.exec_time_ns
        kernel.last_results = res

    out = np.empty((B, NQ, E), dtype=np.float32)
    for b in range(B):
        out[b] = res.results[2 * b]["out"] + res.results[2 * b + 1]["out"] + bo
    return out


# revision 29
# speedup vs baseline: 1.0184x; 1.0184x over previous
"""Trainium2 Bass kernel for nn_CrossAttention (B=4, N=2048, E=768, H=8).

Sharding: 8 cores = 4 batches x 2 head-groups (4 heads of 96 dims each).
Each core computes its batch's attention for its 4 heads plus the partial
output projection; the host sums the two head-group partials per batch and
adds bo.

Per-core dataflow (all feature-major, no on-device transposes):
  K^T_h [96,2048] = Wk_h @ x_kv^T        (lhsT = Wk^T e-tiles, rhs = x_kv^T)
  Q^T_h [96,2048] = Wq_h @ x_q^T
  V     [128tok,4,97] tiles (col 96 = ones -> rowsums ride along matmul)
  S^T   [128kv,1024q] = K_h @ Q_h^T      (lhsT = K^T slice, rhs = Q^T)
  P^T   = exp(S^T/sqrt(768))             (ScalarE, PSUM->SBUF)
  O^T   [97,1024] += V_ext^T @ P^T       (lhsT = V tile, rhs = P^T)
  attn^T = O^T[0:96] * recip(bcast(O^T[96]))   (GpSimd bcast + DVE)
  out   [128q,768] += attn^T_h.T @ Wo^T_h  (partial; host adds group pairs + bo)

v2: single software-pipelined instruction stream.  The attention inner loop
is paced by ScalarE's exp; projection and output-projection matmul groups
are drained from a background queue inside the loop so the PE never idles
(keeps HAM at full clock).  Normalization uses a GpSimd partition-broadcast
of the rowsum row instead of PE ones-matmuls, freeing PSUM and PE cycles.
x tiles are loaded per (e, half) chunk so the first projection matmul can
start ~2us into the kernel.
"""

import os
import sys
import types

import numpy as np

# ---------------------------------------------------------------------------
# NTFF profile hook (the agent image's antenv lacks axon_hooks; degrade OK)
# ---------------------------------------------------------------------------
def _install_ntff_hook():
    if "antenv.axon_hooks" in sys.modules:
        return
    try:
        hooks = types.ModuleType("antenv.axon_hooks")
        hooks._hook = None
        hooks.set_axon_ntff_profile_hook = lambda h: setattr(hooks, "_hook", h)
        hooks.get_axon_ntff_profile_hook = lambda: hooks._hook
        sys.modules["antenv.axon_hooks"] = hooks
        import antenv

        antenv.axon_hooks = hooks
        from trn_agent_boot.trn_boot import _ntff_profile_via_ctypes

        so = "/opt/axon/libaxon_pjrt.so"
        if os.path.exists(so):
            hooks.set_axon_ntff_profile_hook(_ntff_profile_via_ctypes(so))
    except Exception:
        pass


_install_ntff_hook()

import concourse.bacc as bacc
import concourse.tile as tile
import concourse.mybir as mybir
from concourse import bass_utils
from concourse.alu_op_type import AluOpType

F32 = mybir.dt.float32
BF16 = mybir.dt.bfloat16

B = 4
NQ = 2048
NKV = 2048
E = 768
H_LOCAL = 4  # heads per core
HD = 96  # head dim
D = H_LOCAL * HD  # 384 local proj dim
ET = E // 128  # 6 contraction tiles
KV_T = NKV // 128  # 16 kv tiles
INV_SQRT_E = 1.0 / float(np.sqrt(np.float32(E)))

# rowsum broadcast mode: "gpsimd" (idle engine) or "pe" (ones-matmul)
BCAST = os.environ.get("KERNEL_BCAST", "gpsimd")


def build_nc():
    nc = bacc.Bacc("TRN2", target_bir_lowering=False, debug=False)

    xq_t = nc.dram_tensor("xq_t", [E, NQ], BF16, kind="ExternalInput")
    xkv_t = nc.dram_tensor("xkv_t", [E, NKV], BF16, kind="ExternalInput")
    wq_t = nc.dram_tensor("wq_t", [E, D], BF16, kind="ExternalInput")
    wk_t = nc.dram_tensor("wk_t", [E, D], BF16, kind="ExternalInput")
    wv_t = nc.dram_tensor("wv_t", [E, D], BF16, kind="ExternalInput")
    wo_t = nc.dram_tensor("wo_t", [D, E], BF16, kind="ExternalInput")
    bq = nc.dram_tensor("bq", [D], F32, kind="ExternalInput")
    bk = nc.dram_tensor("bk", [D], F32, kind="ExternalInput")
    bv = nc.dram_tensor("bv", [D], F32, kind="ExternalInput")
    out = nc.dram_tensor("out", [NQ, E], F32, kind="ExternalOutput")

    with tile.TileContext(nc) as tc:
        with (
            nc.allow_low_precision(reason="bf16 matmul operands"),
            tc.tile_pool(name="persist", bufs=1) as persist,
            tc.tile_pool(name="work", bufs=1) as work,
            tc.tile_pool(name="psum_pf", bufs=2, space="PSUM") as ppf,
            tc.tile_pool(name="psum_s", bufs=2, space="PSUM") as pps,
            tc.tile_pool(name="psum_o", bufs=1, space="PSUM") as ppo,
        ):
            # ---------------- persistent SBUF ----------------
            xkv_sb = persist.tile([128, ET, 2, 1024], BF16)
            xq_sb = persist.tile([128, ET, 2, 1024], BF16)
            wq_sb = persist.tile([128, ET, D], BF16, tag="wq")
            wk_sb = persist.tile([128, ET, D], BF16, tag="wk")
            wv_sb = persist.tile([128, ET, D], BF16, tag="wv")
            wo_sb = persist.tile([HD, H_LOCAL, E], BF16)
            bq_sb = persist.tile([128, 3], F32)
            bk_sb = persist.tile([128, 3], F32)
            bv_sb = persist.tile([128, D], F32)
            KT = persist.tile([HD, H_LOCAL, NKV], BF16)
            QT = persist.tile([HD, H_LOCAL, NQ], BF16)
            V = persist.tile([128, KV_T, H_LOCAL, HD + 1], BF16)
            attn = persist.tile([HD, H_LOCAL, NQ], BF16)
            ones_f32 = persist.tile([128, HD], F32)
            ones_bf = persist.tile([1, HD], BF16)

            # ---------------- input DMAs (order = priority) ----------------
            nc.sync.dma_start(wk_sb[:], wk_t[:].rearrange("(t p) n -> p t n", p=128))
            nc.sync.dma_start(bq_sb[:], bq[:].rearrange("(j p) -> p j", p=128))
            nc.sync.dma_start(bk_sb[:], bk[:].rearrange("(j p) -> p j", p=128))
            nc.sync.dma_start(bv_sb[:], bv[:].partition_broadcast(128))
            for e in range(ET):
                nc.sync.dma_start(
                    xkv_sb[:, e, 0, :], xkv_t[e * 128 : (e + 1) * 128, 0:1024]
                )
            nc.sync.dma_start(wq_sb[:], wq_t[:].rearrange("(t p) n -> p t n", p=128))
            nc.sync.dma_start(wv_sb[:], wv_t[:].rearrange("(t p) n -> p t n", p=128))
            for e in range(ET):
                nc.sync.dma_start(
                    xkv_sb[:, e, 1, :], xkv_t[e * 128 : (e + 1) * 128, 1024:2048]
                )
            for e in range(ET):
                nc.sync.dma_start(
                    xq_sb[:, e, 0, :], xq_t[e * 128 : (e + 1) * 128, 0:1024]
                )
            # xq half-1 is not needed until the qc1 regions (~100us in);
            # emitted after region(0,0) so it doesn't steal ramp bandwidth
            nc.sync.dma_start(wo_sb[:], wo_t[:].rearrange("(h p) n -> p h n", p=HD))

            # ones column for rowsum-via-matmul + exp-table warm dummy
            nc.vector.memset(ones_f32[:], 1.0)
            nc.vector.tensor_copy(ones_bf[:], ones_f32[0:1, :])
            nc.vector.tensor_copy(
                V[:, :, :, HD : HD + 1],
                ones_f32[:, 0 : KV_T * H_LOCAL].rearrange(
                    "p (t h one) -> p t h one", t=KV_T, h=H_LOCAL, one=1
                ),
            )
            warm = work.tile([128, HD], BF16, tag="warm", bufs=1)
            nc.scalar.activation(
                warm[:], ones_f32[:], mybir.ActivationFunctionType.Exp, scale=0.5
            )

            # ---------------- compute group helpers ----------------
            def proj_col(x_sb, w_sb, b_sb, dst, c):
                # stacked projection: one 512-col chunk of x, all 4 heads.
                # 3 full-width (M=128) matmul groups over the flat 384 output
                # dims (25% fewer PE cycles than 4 per-head M=96 groups),
                # then 6 small DMAs shuffle the stacked rows into the
                # per-head [96, h, n] layout the S-matmuls consume.
                half, n = c // 2, c % 2
                stk = work.tile([128, 3, 512], BF16, tag="stk", bufs=2)
                for j in range(3):
                    ps = ppf.tile([128, 512], F32, tag="pf")
                    for e in range(ET):
                        nc.tensor.matmul(
                            ps[:],
                            w_sb[:, e, j * 128 : (j + 1) * 128],
                            x_sb[:, e, half, n * 512 : (n + 1) * 512],
                            start=(e == 0),
                            stop=(e == ET - 1),
                        )
                    nc.vector.tensor_scalar_add(
                        out=stk[:, j, :], in0=ps[:], scalar1=b_sb[:, j : j + 1]
                    )
                cs = c * 512
                for dst_sl, j, src_lo, src_hi in (
                    ((0, 96, 0), 0, 0, 96),
                    ((0, 32, 1), 0, 96, 128),
                    ((32, 96, 1), 1, 0, 64),
                    ((0, 64, 2), 1, 64, 128),
                    ((64, 96, 2), 2, 0, 32),
                    ((0, 96, 3), 2, 32, 128),
                ):
                    lo, hi, h = dst_sl
                    nc.sync.dma_start(
                        dst[lo:hi, h, cs : cs + 512], stk[src_lo:src_hi, j, :]
                    )

            def v_group(tg):
                half, t = tg // 8, tg % 8
                ps = ppf.tile([128, 512], F32, tag="pf")
                for e in range(ET):
                    nc.tensor.matmul(
                        ps[:, 0:D],
                        xkv_sb[:, e, half, t * 128 : (t + 1) * 128],
                        wv_sb[:, e, :],
                        start=(e == 0),
                        stop=(e == ET - 1),
                    )
                nc.vector.tensor_tensor(
                    out=V[:, tg, :, 0:HD],
                    in0=ps[:, 0:D].rearrange("p (h d) -> p h d", h=H_LOCAL),
                    in1=bv_sb.rearrange("p (h d) -> p h d", h=H_LOCAL),
                    op=AluOpType.add,
                )

            def outproj_group(qc, t):
                qt = qc * 8 + t
                fa = ppf.tile([128, 512], F32, tag="pf")
                fb = ppf.tile([128, 512], F32, tag="pf")
                for h in range(H_LOCAL):
                    nc.tensor.matmul(
                        fa[:],
                        attn[:, h, qt * 128 : (qt + 1) * 128],
                        wo_sb[:, h, 0:512],
                        start=(h == 0),
                        stop=(h == H_LOCAL - 1),
                    )
                    nc.tensor.matmul(
                        fb[:, 0:256],
                        attn[:, h, qt * 128 : (qt + 1) * 128],
                        wo_sb[:, h, 512:768],
                        start=(h == 0),
                        stop=(h == H_LOCAL - 1),
                    )
                ob = work.tile([128, E], F32, tag="ob", bufs=2)
                nc.vector.tensor_copy(ob[:, 0:512], fa[:])
                nc.vector.tensor_copy(ob[:, 512:768], fb[:, 0:256])
                nc.sync.dma_start(out[qt * 128 : (qt + 1) * 128, :], ob[:])

            # ---------------- upfront projections ----------------
            # One stacked K column and the two qc0 Q columns give every head
            # enough to start attention ~15us in; everything else becomes
            # background work drained inside the attention regions.
            proj_col(xkv_sb, wk_sb, bk_sb, KT, 0)
            proj_col(xq_sb, wq_sb, bq_sb, QT, 0)
            proj_col(xq_sb, wq_sb, bq_sb, QT, 1)
            for tg in range(4):
                v_group(tg)

            bg = [
                ("k", 1),
                ("v", 4),
                ("v", 5),
                ("v", 6),
                ("v", 7),
                ("k", 2),
                ("v", 8),
                ("v", 9),
                ("k", 3),
            ] + [("v", tg) for tg in range(10, KV_T)]
            bg_late = [("q", 2), ("q", 3)]

            def drain_bg(k):
                while k > 0 and bg:
                    item = bg.pop(0)
                    if item[0] == "v":
                        v_group(item[1])
                    elif item[0] == "k":
                        proj_col(xkv_sb, wk_sb, bk_sb, KT, item[1])
                    elif item[0] == "q":
                        proj_col(xq_sb, wq_sb, bq_sb, QT, item[1])
                    else:
                        _, qc, t = item
                        outproj_group(qc, t)
                    k -= 1

            # ---------------- attention regions ----------------
            def norm_chunk(qc, h, o_sb, col, width, attn_col=None):
                if attn_col is None:
                    attn_col = col
                # o_sb rows 0:96 = O^T, row 96 = rowsums for cols [col,col+width)
                # of this (qc,h) q-chunk.  Engines can't shift partitions, so
                # DMA the rowsum row down to partition 0, broadcast it across
                # 96 partitions on the idle GpSimd engine, then recip+mul.
                rs = work.tile([1, 1024], F32, tag="rsrow", bufs=2)
                nc.sync.dma_start(rs[0:1, 0:width], o_sb[HD : HD + 1, col : col + width])
                rb_raw = work.tile([HD, 1024], F32, tag="rbr", bufs=2)
                if BCAST == "gpsimd":
                    nc.gpsimd.partition_broadcast(
                        rb_raw[:, 0:width], rs[0:1, 0:width]
                    )
                else:
                    rs_bf = work.tile([1, 1024], BF16, tag="rsbf", bufs=2)
                    nc.vector.tensor_copy(rs_bf[0:1, 0:width], rs[0:1, 0:width])
                    for n in range(width // 512):
                        bc = ppf.tile([128, 512], F32, tag="pf")
                        nc.tensor.matmul(
                            bc[0:HD, :],
                            ones_bf[:],
                            rs_bf[:, n * 512 : (n + 1) * 512],
                            start=True,
                            stop=True,
                        )
                        nc.vector.tensor_copy(
                            rb_raw[:, n * 512 : (n + 1) * 512], bc[0:HD, :]
                        )
                rb_inv = work.tile([HD, 1024], F32, tag="rbi", bufs=2)
                rscr = work.tile([HD, 1024], F32, tag="rsc", bufs=2)
                nc.vector.reciprocal_approx_accurate(
                    out=rb_inv[:, 0:width],
                    in_=rb_raw[:, 0:width],
                    scratch=rscr[:, 0:width],
                )
                nc.vector.tensor_mul(
                    attn[:, h, qc * 1024 + attn_col : qc * 1024 + attn_col + width],
                    o_sb[0:HD, col : col + width],
                    rb_inv[:, 0:width],
                )

            def region(qc, h, bg_rate, split_norm=False):
                po = ppo.tile([HD + 1, 1024], F32, tag="o")
                for kv in range(KV_T):
                    s = pps.tile([128, 1024], F32, tag="s")
                    for n in range(2):
                        nc.tensor.matmul(
                            s[:, n * 512 : (n + 1) * 512],
                            KT[:, h, kv * 128 : (kv + 1) * 128],
                            QT[:, h, qc * 1024 + n * 512 : qc * 1024 + (n + 1) * 512],
                            start=True,
                            stop=True,
                        )
                    p = work.tile([128, 1024], BF16, tag="p", bufs=3)
                    nc.scalar.activation(
                        p[:], s[:], mybir.ActivationFunctionType.Exp, scale=INV_SQRT_E
                    )
                    for n in range(2):
                        nc.tensor.matmul(
                            po[:, n * 512 : (n + 1) * 512],
                            V[:, kv, h, :],
                            p[:, n * 512 : (n + 1) * 512],
                            start=(kv == 0),
                            stop=(kv == KV_T - 1),
                        )
                    num, den = bg_rate
                    if den and kv % den == den - 1:
                        drain_bg(num)
                o_sb = work.tile([HD + 1, 1024], F32, tag="osb", bufs=2)
                if split_norm:
                    # halve the tail latency: normalize cols 0:512 while the
                    # copy of cols 512:1024 is still in flight
                    nc.vector.tensor_copy(o_sb[:, 0:512], po[:, 0:512])
                    norm_chunk(qc, h, o_sb, 0, 512)
                    nc.vector.tensor_copy(o_sb[:, 512:1024], po[:, 512:1024])
                    norm_chunk(qc, h, o_sb, 512, 512)
                else:
                    nc.vector.tensor_copy(o_sb[:], po[:])
                    norm_chunk(qc, h, o_sb, 0, 1024)

            # qc0: heads 0-3 with proj background work
            region(0, 0, (1, 1))
            for e in range(ET):
                nc.sync.dma_start(
                    xq_sb[:, e, 1, :], xq_t[e * 128 : (e + 1) * 128, 1024:2048]
                )
            bg.extend(bg_late)
            region(0, 1, (1, 8))
            region(0, 2, (0, 0))
            region(0, 3, (0, 0))
            # qc0 output projection rides through the qc1 regions
            for t in range(8):
                bg.append(("op", 0, t))
            region(1, 0, (1, 8))
            region(1, 1, (1, 8))
            region(1, 2, (1, 8))

            # ---- final region (1,3): two 512-wide half-regions ----
            # The closing normalization chain (~5us of cross-engine latency)
            # cannot be hidden by the scheduler once all other work has
            # drained, so split the last region: half-a's normalization
            # hides under half-b's kv loop, and half-b's normalization hides
            # under half-a's output-projection tiles.
            def half_region(hb):
                cs = 1024 + hb * 512
                po = ppo.tile([HD + 1, 1024], F32, tag="o")
                for kv in range(KV_T):
                    s = pps.tile([128, 1024], F32, tag="s")
                    nc.tensor.matmul(
                        s[:, 0:512],
                        KT[:, 3, kv * 128 : (kv + 1) * 128],
                        QT[:, 3, cs : cs + 512],
                        start=True,
                        stop=True,
                    )
                    p = work.tile([128, 1024], BF16, tag="p", bufs=3)
                    nc.scalar.activation(
                        p[:, 0:512],
                        s[:, 0:512],
                        mybir.ActivationFunctionType.Exp,
                        scale=INV_SQRT_E,
                    )
                    nc.tensor.matmul(
                        po[:, 0:512],
                        V[:, kv, 3, :],
                        p[:, 0:512],
                        start=(kv == 0),
                        stop=(kv == KV_T - 1),
                    )
                    drains = (5, 11) if hb == 0 else (6, 10, 13)
                    if kv in drains and bg:
                        # leftover outproj tiles interleave with the thin
                        # kv loops to fill the ScalarE-paced PE slack
                        item = bg.pop(0)
                        outproj_group(item[1], item[2])
                o_sb = work.tile([HD + 1, 1024], F32, tag="osb", bufs=2)
                nc.vector.tensor_copy(o_sb[:, 0:512], po[:, 0:512])
                norm_chunk(1, 3, o_sb, 0, 512, attn_col=hb * 512)

            half_region(0)
            # queue the half-a output tiles; t0-t3 only need attn cols
            # 1024:1536 (half-a) of head 3
            bg = [("op", 1, t) for t in range(4)] + bg
            half_region(1)
            drain_bg(len(bg))
            for t in range(4, 8):
                outproj_group(1, t)

    nc.compile()
    return nc


_NC_CACHE = None


def kernel(x_query, x_kv, Wq, bq, Wk, bk, Wv, bv, Wo, bo):
    global _NC_CACHE
    x_query = np.asarray(x_query, dtype=np.float32)
    x_kv = np.asarray(x_kv, dtype=np.float32)
    Wq = np.asarray(Wq, dtype=np.float32)
    Wk = np.asarray(Wk, dtype=np.float32)
    Wv = np.asarray(Wv, dtype=np.float32)
    Wo = np.asarray(Wo, dtype=np.float32)
    bq = np.asarray(bq, dtype=np.float32)
    bk = np.asarray(bk, dtype=np.float32)
    bv = np.asarray(bv, dtype=np.float32)
    bo = np.asarray(bo, dtype=np.float32)

    if _NC_CACHE is None:
        _NC_CACHE = build_nc()
    nc = _NC_CACHE

    import ml_dtypes

    xdt_np = ml_dtypes.bfloat16

    in_maps = []
    for c in range(8):
        b, g = divmod(c, 2)
        sl = slice(g * D, (g + 1) * D)
        in_maps.append(
            {
                "xq_t": np.ascontiguousarray(x_query[b].T).astype(xdt_np),
                "xkv_t": np.ascontiguousarray(x_kv[b].T).astype(xdt_np),
                "wq_t": np.ascontiguousarray(Wq[sl, :].T).astype(xdt_np),
                "wk_t": np.ascontiguousarray(Wk[sl, :].T).astype(xdt_np),
                "wv_t": np.ascontiguousarray(Wv[sl, :].T).astype(xdt_np),
                "wo_t": np.ascontiguousarray(Wo[:, sl].T).astype(xdt_np),
                "bq": np.ascontiguousarray(bq[sl]),
                "bk": np.ascontiguousarray(bk[sl]),
                "bv": np.ascontiguousarray(bv[sl]),
            }
        )

    trace = bool(int(os.environ.get("KERNEL_TRACE", "0")))
    res = bass_utils.run_bass_kernel_spmd(
        nc, in_maps, core_ids=list(range(8)), trace=trace
    )
    if trace:
        kernel.last_exec_time_ns = res.exec_time_ns
        kernel.last_results = res

    out = np.empty((B, NQ, E), dtype=np.float32)
    for b in range(B):
        out[b] = res.results[2 * b]["out"] + res.results[2 * b + 1]["out"] + bo
    return out
